# revision 22
# baseline (speedup 1.0000x reference)
"""BatchAllTripletLoss on 8 Trainium2 NeuronCores (pair-packed version).

Contract: kernel(**inputs) takes the FULL inputs (embs [512,128] f32,
idtys [512] int64) and returns the FULL output (scalar f32 loss).

Math: d = pairwise euclidean distances [512,512];
  loss = sum_{a,p,n} relu(d[a,p]-d[a,n]+margin)*mask / (num_pos + eps)
The mask factorizes as pos[a,p]*neg[a,n]. All idtys-derived index
bookkeeping happens on the host (pure bookkeeping, no embs math): the
~3930 valid (a,p) pairs are packed into 128-row tiles, split
4 anchor-groups x 2 cores. Per core (AH=128 anchors, NT tiles of 128
pair slots):

 1. dneg[a,n] = sqrt(d2[a,n] + BIGSQ*same[a,n]) via THREE bf16
    matmuls accumulated in PSUM: -2*A^T E (K=128), a K=64 group-one-hot
    fold ohA*BIGSQ x ohN (the same-id mask), and a K=33 norm fold
    carrying the sqa/sq rank-1 terms (rows written on device at legal
    partition starts 64/96), then one ACT sqrt. Same-id columns land
    at ~1e6 so relu/count see 0 there.
 2. Pair distances: host stages gathered anchor/positive embeddings
    A,P [D, NT*128]; d2_pair = colsum((A-P)^2) via per-tile matmuls
    with the squared-diff block as the STATIONARY operand and ones as
    the moving operand -- lands directly as [128, NT] columns in PSUM.
    One ACT sqrt + mask -> x [128, NT].
 3. Main loop over NT tiles: PE one-hot matmul broadcasts each pair's
    dneg row into PSUM [128,512]; ACT computes relu(x - dneg) with a
    fused accum_out row-reduction; DVE counts positives via is_gt with
    fused accum_out. No gpsimd compute, no indirect DMA.
Inputs ride 4 merged DMAs split over the two HWDGE rings (sync and
scalar); each DMA costs ~2.0-2.8us issue-to-completion (HBM receipt
latency), so count and placement matter more than bytes. Keep DMA
partition counts at 128 -- odd counts (e.g. 97) hit an ~8us slow path.
Per-core output [128, 2*NT] partials; host reduces and divides.
"""

import numpy as np

B = 512
D = 128
NCORES = 8
AH = 128          # anchors per core
MARGIN = 0.2
BIGSQ = 1.0e12    # added to d2 on same-id columns; sqrt -> ~1e6

_CACHE = {}


def _build_bass(nt):
    import concourse.bass as bass
    import concourse.tile as tile
    from concourse import mybir

    f32 = mybir.dt.float32
    bf16 = mybir.dt.bfloat16
    AF = mybir.ActivationFunctionType
    OP = mybir.AluOpType
    X = mybir.AxisListType.X

    NPX = nt * 128

    nc = bass.Bass()
    # emT | emTA
    emTT = nc.dram_tensor("emTT", [D, B + AH], bf16, kind="ExternalInput")
    # fold block, K=128 (device-written rows must start at partition
    # 64/96; partition count kept at 128 -- odd counts hit a slow DMA
    # path): cols 0:B = rhs (rows 0:64 ohN, row 64 sq[dev], row 96
    # ones), cols B:B+AH = lhsT (rows 0:64 BIGSQ*ohA, row 64 ones,
    # row 96 sqa[dev]); rows 65:96 and 97:128 zero padding
    fold = nc.dram_tensor("fold", [128, B + AH], bf16, kind="ExternalInput")
    # anchor | positive embedding per pair slot
    appp = nc.dram_tensor("appp", [D, 2 * NPX], bf16, kind="ExternalInput")
    # slot->anchor one-hot | slot valid mask
    wvm = nc.dram_tensor("wvm", [AH, NPX + nt], bf16, kind="ExternalInput")
    out = nc.dram_tensor("out", [128, 2 * nt], f32, kind="ExternalOutput")

    with tile.TileContext(nc) as tc:
        with (
            tc.tile_pool(name="sb", bufs=1) as sb,
            tc.tile_pool(name="psd2", bufs=1, space="PSUM") as psd2,
            tc.tile_pool(name="pspx", bufs=1, space="PSUM") as pspx,
            tc.tile_pool(name="psdn", bufs=2, space="PSUM") as psdn,
            tc.tile_pool(name="junka", bufs=4) as junka,
            tc.tile_pool(name="junkc", bufs=4) as junkc,
        ):
            emTT_t = sb.tile([D, B + AH], bf16)
            fold_t = sb.tile([128, B + AH], bf16)
            appp_t = sb.tile([D, 2 * NPX], bf16)
            wvm_t = sb.tile([AH, NPX + nt], bf16)
            nc.sync.dma_start(out=emTT_t[:], in_=emTT[:])
            nc.sync.dma_start(out=fold_t[:], in_=fold[:])
            nc.scalar.dma_start(out=appp_t[:], in_=appp[:])
            nc.sync.dma_start(out=wvm_t[:], in_=wvm[:])
            emT_s = emTT_t[:, 0:B]
            emTA_s = emTT_t[:, B : B + AH]
            AP_s = appp_t[:, 0:NPX]
            PP_s = appp_t[:, NPX : 2 * NPX]
            W_s = wvm_t[:, 0:NPX]
            vm_s = wvm_t[:, NPX : NPX + nt]

            # one ones tile serves as: ones column (pair-d2 reduces),
            # [D, AH] all-ones stationary (sq broadcast), [D, B] moving
            # all-ones (sqa broadcast)
            ones_db = sb.tile([D, B], bf16)
            nc.vector.memset(ones_db[:], 1.0)
            ones_col = ones_db[:, 0:1]

            e2 = sb.tile([D, B], bf16)
            nc.vector.tensor_mul(e2[:], emT_s, emT_s)
            e2a = sb.tile([D, AH], bf16)
            nc.vector.tensor_mul(e2a[:], emTA_s, emTA_s)
            emTAm2 = sb.tile([D, AH], bf16)
            nc.vector.tensor_scalar_mul(emTAm2[:], emTA_s, -2.0)

            # ---- dneg = sqrt(sq + sqa - 2*dot + BIGSQ*same)  [AH, B]
            # all four terms fold straight into one PSUM accumulation, no
            # PSUM->SBUF round trips: sq[n] = ONES^T x e2 (all-ones
            # stationary broadcasts the column sums to every anchor row),
            # sqa[a] = e2a^T x ONES (broadcasts anchor norms along n)
            ps_d2 = psd2.tile([AH, B], f32)
            nc.tensor.matmul(
                ps_d2[:], ones_db[:, 0:AH], e2[:], start=True, stop=False
            )
            nc.tensor.matmul(ps_d2[:], e2a[:], ones_db[:], start=False, stop=False)
            nc.tensor.matmul(ps_d2[:], emTAm2[:], emT_s, start=False, stop=False)
            nc.tensor.matmul(
                ps_d2[:], fold_t[0:64, B : B + AH], fold_t[0:64, 0:B],
                start=False, stop=True,
            )
            dneg_b = sb.tile([AH, B], bf16)
            nc.scalar.activation(dneg_b[:], ps_d2[:], AF.Sqrt)

            # ---- pair distances -> x = (d[a,p]+margin)*valid  [128, nt]
            dfb = sb.tile([D, NPX], bf16)
            nc.vector.tensor_sub(dfb[:], AP_s, PP_s)
            dfq = sb.tile([D, NPX], bf16)
            nc.vector.tensor_mul(dfq[:], dfb[:], dfb[:])
            ps_px = pspx.tile([128, nt], f32)
            for t in range(nt):
                nc.tensor.matmul(
                    ps_px[:, t : t + 1], dfq[:, t * 128 : (t + 1) * 128],
                    ones_col[:], start=True, stop=True,
                )
            xg = sb.tile([128, nt], f32)
            nc.scalar.activation(xg[:], ps_px[:], AF.Sqrt)
            x_sb = sb.tile([128, nt], f32)
            nc.vector.scalar_tensor_tensor(
                out=x_sb[:], in0=xg[:], scalar=MARGIN, in1=vm_s,
                op0=OP.add, op1=OP.mult,
            )

            # ---- main loop: per tile, broadcast dneg rows to pair slots,
            # relu-sum on ACT (fused row-reduce), count on DVE (fused)
            accR = sb.tile([128, nt], f32)
            accC = sb.tile([128, nt], f32)
            for t in range(nt):
                ps_dn = psdn.tile([128, B], f32, tag="dn")
                nc.tensor.matmul(
                    ps_dn[:], W_s[:, t * 128 : (t + 1) * 128], dneg_b[:],
                    start=True, stop=True,
                )
                tt = junka.tile([128, B], bf16)
                nc.scalar.activation(
                    tt[:], ps_dn[:], AF.Relu, bias=x_sb[:, t : t + 1],
                    scale=-1.0, accum_out=accR[:, t : t + 1],
                )
                gg = junkc.tile([128, B], bf16)
                nc.vector.tensor_scalar(
                    out=gg[:], in0=tt[:], scalar1=0.0, scalar2=None,
                    op0=OP.is_gt, op1=OP.add,
                    accum_out=accC[:, t : t + 1],
                )

            # accR is ready ~0.7us before accC (relu accum vs trailing
            # is_gt); put the later, exec-gating accC transfer on the
            # fast sync ring and accR on the slow scalar ring
            nc.scalar.dma_start(out=out[:, 0:nt], in_=accR[:])
            nc.sync.dma_start(out=out[:, nt : 2 * nt], in_=accC[:])

    return nc


def _legalize_waits(bir: bytes) -> bytes:
    """walrus codegen in this toolchain allows only one sync-wait per
    instruction; split extra waits into standalone EventSemaphore insts."""
    import json

    m = json.loads(bir)
    for fn in m["functions"]:
        for bb in fn["blocks"]:
            new = []
            for inst in bb["instructions"]:
                si = inst.get("sync_info")
                if si and si.get("on_wait") and len(si["on_wait"]) > 1:
                    waits = si["on_wait"]
                    for j, w in enumerate(waits[:-1]):
                        new.append(
                            {
                                "engine": inst["engine"],
                                "ins": [],
                                "outs": [],
                                "name": f"{inst['name']}-w{j}",
                                "opcode": "EventSemaphore",
                                "sync_info": {"on_update": [], "on_wait": [w]},
                            }
                        )
                    si["on_wait"] = [waits[-1]]
                new.append(inst)
            bb["instructions"] = new
    return json.dumps(m).encode()


def _get_nc(nt):
    key = ("nc", nt)
    if key not in _CACHE:
        nc = _build_bass(nt)
        orig = nc.to_json_bytes
        nc.to_json_bytes = lambda: _legalize_waits(orig())
        _CACHE[key] = nc
    return _CACHE[key]


def _plan(ids):
    """Per-core pair lists: 4 anchor groups x 2 cores, interleaved split."""
    ids = np.asarray(ids).astype(np.int64)
    plans = []
    for g in range(4):
        a0 = g * AH
        pairs = []
        for la in range(AH):
            a = a0 + la
            for p in np.where(ids == ids[a])[0]:
                if p != a:
                    pairs.append((la, int(p)))
        plans.append(pairs[0::2])
        plans.append(pairs[1::2])
    nt = max(1, -(-max(len(p) for p in plans) // 128))
    return plans, nt


def make_in_maps(embs: np.ndarray, idtys: np.ndarray):
    import ml_dtypes

    bf = ml_dtypes.bfloat16
    embs = np.ascontiguousarray(np.asarray(embs, dtype=np.float32))
    ids = np.asarray(idtys).astype(np.int64)
    plans, nt = _plan(ids)
    NPX = nt * 128
    emT = embs.T
    ohN = np.zeros((64, B), np.float32)
    ohN[ids, np.arange(B)] = 1.0
    in_maps = []
    for c in range(NCORES):
        a0 = (c // 2) * AH
        pairs = plans[c]
        npc = len(pairs)
        la = np.zeros(NPX, np.int64)
        pp = np.full(NPX, a0, np.int64)
        if npc:
            la[:npc] = [q[0] for q in pairs]
            pp[:npc] = [q[1] for q in pairs]

        emTT = np.concatenate([emT, emT[:, a0 : a0 + AH]], axis=1)

        fold = np.zeros((128, B + AH), np.float32)
        fold[0:64, 0:B] = ohN                      # rhs: ohN
        ohA = np.zeros((64, AH), np.float32)
        ohA[ids[a0 : a0 + AH], np.arange(AH)] = BIGSQ
        fold[0:64, B : B + AH] = ohA               # lhsT: BIGSQ*ohA

        appp = np.concatenate([embs[a0 + la].T, embs[pp].T], axis=1)

        wvm = np.zeros((AH, NPX + nt), np.float32)
        wvm[la, np.arange(NPX)] = 1.0
        vm = np.zeros(NPX, np.float32)
        vm[:npc] = 1.0
        wvm[:, NPX : NPX + nt] = vm.reshape(nt, 128).T

        in_maps.append(
            {
                "emTT": np.ascontiguousarray(emTT.astype(bf)),
                "fold": np.ascontiguousarray(fold.astype(bf)),
                "appp": np.ascontiguousarray(appp.astype(bf)),
                "wvm": np.ascontiguousarray(wvm.astype(bf)),
            }
        )
    return in_maps, nt


def combine(results, nt):
    total = 0.0
    count = 0.0
    for r in results:
        o = np.asarray(r["out"], dtype=np.float64)
        total += o[:, 0:nt].sum()
        count += o[:, nt:].sum()
    loss = np.float32(total / (count + 1e-16))
    return np.array(loss, dtype=np.float32)


def kernel(embs: np.ndarray, idtys: np.ndarray) -> np.ndarray:
    from concourse import bass_utils

    in_maps, nt = make_in_maps(np.asarray(embs), np.asarray(idtys))
    nc = _get_nc(nt)
    res = bass_utils.run_bass_kernel_spmd(nc, in_maps, list(range(NCORES)))
    return combine(res.results, nt)


# revision 23
# speedup vs baseline: 1.0193x; 1.0193x over previous
"""BatchAllTripletLoss on 8 Trainium2 NeuronCores (pair-packed version).

Contract: kernel(**inputs) takes the FULL inputs (embs [512,128] f32,
idtys [512] int64) and returns the FULL output (scalar f32 loss).

Math: d = pairwise euclidean distances [512,512];
  loss = sum_{a,p,n} relu(d[a,p]-d[a,n]+margin)*mask / (num_pos + eps)
The mask factorizes as pos[a,p]*neg[a,n]. All idtys-derived index
bookkeeping happens on the host (pure bookkeeping, no embs math): the
~3930 valid (a,p) pairs are packed into 128-row tiles, split
4 anchor-groups x 2 cores. Per core (AH=128 anchors, NT tiles of 128
pair slots):

 1. dneg[a,n] = sqrt(d2[a,n] + BIGSQ*same[a,n]) via THREE bf16
    matmuls accumulated in PSUM: -2*A^T E (K=128), a K=64 group-one-hot
    fold ohA*BIGSQ x ohN (the same-id mask), and a K=33 norm fold
    carrying the sqa/sq rank-1 terms (rows written on device at legal
    partition starts 64/96), then one ACT sqrt. Same-id columns land
    at ~1e6 so relu/count see 0 there.
 2. Pair distances: host stages gathered anchor/positive embeddings
    A,P [D, NT*128]; d2_pair = colsum((A-P)^2) via per-tile matmuls
    with the squared-diff block as the STATIONARY operand and ones as
    the moving operand -- lands directly as [128, NT] columns in PSUM.
    One ACT sqrt + mask -> x [128, NT].
 3. Main loop over NT tiles: PE one-hot matmul broadcasts each pair's
    dneg row into PSUM [128,512]; ACT computes relu(x - dneg) with a
    fused accum_out row-reduction; DVE counts positives via is_gt with
    fused accum_out. No gpsimd compute, no indirect DMA.
Inputs ride 4 merged DMAs split over the two HWDGE rings (sync and
scalar); each DMA costs ~2.0-2.8us issue-to-completion (HBM receipt
latency), so count and placement matter more than bytes. Keep DMA
partition counts at 128 -- odd counts (e.g. 97) hit an ~8us slow path.
Per-core output [128, 2*NT] partials; host reduces and divides.
"""

import numpy as np

B = 512
D = 128
NCORES = 8
AH = 128          # anchors per core
MARGIN = 0.2
BIGSQ = 1.0e12    # added to d2 on same-id columns; sqrt -> ~1e6

_CACHE = {}


def _build_bass(nt):
    import concourse.bass as bass
    import concourse.tile as tile
    from concourse import mybir

    f32 = mybir.dt.float32
    bf16 = mybir.dt.bfloat16
    AF = mybir.ActivationFunctionType
    OP = mybir.AluOpType
    X = mybir.AxisListType.X

    NPX = nt * 128

    nc = bass.Bass()
    # emT | emTA
    emTT = nc.dram_tensor("emTT", [D, B + AH], bf16, kind="ExternalInput")
    # fold block, K=128 (device-written rows must start at partition
    # 64/96; partition count kept at 128 -- odd counts hit a slow DMA
    # path): cols 0:B = rhs (rows 0:64 ohN, row 64 sq[dev], row 96
    # ones), cols B:B+AH = lhsT (rows 0:64 BIGSQ*ohA, row 64 ones,
    # row 96 sqa[dev]); rows 65:96 and 97:128 zero padding
    fold = nc.dram_tensor("fold", [128, B + AH], bf16, kind="ExternalInput")
    # anchor | positive embedding per pair slot
    appp = nc.dram_tensor("appp", [D, 2 * NPX], bf16, kind="ExternalInput")
    # slot->anchor one-hot | slot valid mask
    wvm = nc.dram_tensor("wvm", [AH, NPX + nt], bf16, kind="ExternalInput")
    out = nc.dram_tensor("out", [128, 2 * nt], f32, kind="ExternalOutput")

    with tile.TileContext(nc) as tc:
        with (
            tc.tile_pool(name="sb", bufs=1) as sb,
            tc.tile_pool(name="psd2", bufs=1, space="PSUM") as psd2,
            tc.tile_pool(name="pspx", bufs=1, space="PSUM") as pspx,
            tc.tile_pool(name="psdn", bufs=2, space="PSUM") as psdn,
            tc.tile_pool(name="junka", bufs=4) as junka,
            tc.tile_pool(name="junkc", bufs=4) as junkc,
        ):
            emTT_t = sb.tile([D, B + AH], bf16)
            fold_t = sb.tile([128, B + AH], bf16)
            appp_t = sb.tile([D, 2 * NPX], bf16)
            wvm_t = sb.tile([AH, NPX + nt], bf16)
            nc.sync.dma_start(out=emTT_t[:], in_=emTT[:])
            nc.sync.dma_start(out=fold_t[:], in_=fold[:])
            nc.scalar.dma_start(out=appp_t[:], in_=appp[:])
            nc.sync.dma_start(out=wvm_t[:], in_=wvm[:])
            emT_s = emTT_t[:, 0:B]
            emTA_s = emTT_t[:, B : B + AH]
            AP_s = appp_t[:, 0:NPX]
            PP_s = appp_t[:, NPX : 2 * NPX]
            W_s = wvm_t[:, 0:NPX]
            vm_s = wvm_t[:, NPX : NPX + nt]

            # one ones tile serves as: ones column (pair-d2 reduces),
            # [D, AH] all-ones stationary (sq broadcast), [D, B] moving
            # all-ones (sqa broadcast)
            ones_db = sb.tile([D, B], bf16)
            nc.vector.memset(ones_db[:], 1.0)
            ones_col = ones_db[:, 0:1]

            emTAm2 = sb.tile([D, AH], bf16)
            nc.vector.tensor_scalar_mul(emTAm2[:], emTA_s, -2.0)
            e2 = sb.tile([D, B], bf16)
            nc.vector.tensor_mul(e2[:], emT_s, emT_s)
            e2a = sb.tile([D, AH], bf16)
            nc.vector.tensor_mul(e2a[:], emTA_s, emTA_s)

            # ---- dneg = sqrt(sq + sqa - 2*dot + BIGSQ*same)  [AH, B]
            # all four terms fold straight into one PSUM accumulation, no
            # PSUM->SBUF round trips: sq[n] = ONES^T x e2 (all-ones
            # stationary broadcasts the column sums to every anchor row),
            # sqa[a] = e2a^T x ONES (broadcasts anchor norms along n)
            ps_d2 = psd2.tile([AH, B], f32)
            nc.tensor.matmul(ps_d2[:], emTAm2[:], emT_s, start=True, stop=False)
            nc.tensor.matmul(
                ps_d2[:], ones_db[:, 0:AH], e2[:], start=False, stop=False
            )
            nc.tensor.matmul(ps_d2[:], e2a[:], ones_db[:], start=False, stop=False)
            nc.tensor.matmul(
                ps_d2[:], fold_t[0:64, B : B + AH], fold_t[0:64, 0:B],
                start=False, stop=True,
            )
            dneg_b = sb.tile([AH, B], bf16)
            nc.scalar.activation(dneg_b[:], ps_d2[:], AF.Sqrt)

            # ---- pair distances -> x = (d[a,p]+margin)*valid  [128, nt]
            dfb = sb.tile([D, NPX], bf16)
            nc.vector.tensor_sub(dfb[:], AP_s, PP_s)
            dfq = sb.tile([D, NPX], bf16)
            nc.vector.tensor_mul(dfq[:], dfb[:], dfb[:])
            ps_px = pspx.tile([128, nt], f32)
            for t in range(nt):
                nc.tensor.matmul(
                    ps_px[:, t : t + 1], dfq[:, t * 128 : (t + 1) * 128],
                    ones_col[:], start=True, stop=True,
                )
            xg = sb.tile([128, nt], f32)
            nc.scalar.activation(xg[:], ps_px[:], AF.Sqrt)
            x_sb = sb.tile([128, nt], f32)
            nc.vector.scalar_tensor_tensor(
                out=x_sb[:], in0=xg[:], scalar=MARGIN, in1=vm_s,
                op0=OP.add, op1=OP.mult,
            )

            # ---- main loop: per tile, broadcast dneg rows to pair slots,
            # relu-sum on ACT (fused row-reduce), count on DVE (fused)
            accR = sb.tile([128, nt], f32)
            accC = sb.tile([128, nt], f32)
            for t in range(nt):
                ps_dn = psdn.tile([128, B], f32, tag="dn")
                nc.tensor.matmul(
                    ps_dn[:], W_s[:, t * 128 : (t + 1) * 128], dneg_b[:],
                    start=True, stop=True,
                )
                tt = junka.tile([128, B], bf16)
                nc.scalar.activation(
                    tt[:], ps_dn[:], AF.Relu, bias=x_sb[:, t : t + 1],
                    scale=-1.0, accum_out=accR[:, t : t + 1],
                )
                gg = junkc.tile([128, B], bf16)
                nc.vector.tensor_scalar(
                    out=gg[:], in0=tt[:], scalar1=0.0, scalar2=None,
                    op0=OP.is_gt, op1=OP.add,
                    accum_out=accC[:, t : t + 1],
                )

            # accR is ready ~0.7us before accC (relu accum vs trailing
            # is_gt); put the later, exec-gating accC transfer on the
            # fast sync ring and accR on the slow scalar ring
            nc.scalar.dma_start(out=out[:, 0:nt], in_=accR[:])
            nc.sync.dma_start(out=out[:, nt : 2 * nt], in_=accC[:])

    return nc


def _legalize_waits(bir: bytes) -> bytes:
    """walrus codegen in this toolchain allows only one sync-wait per
    instruction; split extra waits into standalone EventSemaphore insts."""
    import json

    m = json.loads(bir)
    for fn in m["functions"]:
        for bb in fn["blocks"]:
            new = []
            for inst in bb["instructions"]:
                si = inst.get("sync_info")
                if si and si.get("on_wait") and len(si["on_wait"]) > 1:
                    waits = si["on_wait"]
                    for j, w in enumerate(waits[:-1]):
                        new.append(
                            {
                                "engine": inst["engine"],
                                "ins": [],
                                "outs": [],
                                "name": f"{inst['name']}-w{j}",
                                "opcode": "EventSemaphore",
                                "sync_info": {"on_update": [], "on_wait": [w]},
                            }
                        )
                    si["on_wait"] = [waits[-1]]
                new.append(inst)
            bb["instructions"] = new
    return json.dumps(m).encode()


def _get_nc(nt):
    key = ("nc", nt)
    if key not in _CACHE:
        nc = _build_bass(nt)
        orig = nc.to_json_bytes
        nc.to_json_bytes = lambda: _legalize_waits(orig())
        _CACHE[key] = nc
    return _CACHE[key]


def _plan(ids):
    """Per-core pair lists: 4 anchor groups x 2 cores, interleaved split."""
    ids = np.asarray(ids).astype(np.int64)
    plans = []
    for g in range(4):
        a0 = g * AH
        pairs = []
        for la in range(AH):
            a = a0 + la
            for p in np.where(ids == ids[a])[0]:
                if p != a:
                    pairs.append((la, int(p)))
        plans.append(pairs[0::2])
        plans.append(pairs[1::2])
    nt = max(1, -(-max(len(p) for p in plans) // 128))
    return plans, nt


def make_in_maps(embs: np.ndarray, idtys: np.ndarray):
    import ml_dtypes

    bf = ml_dtypes.bfloat16
    embs = np.ascontiguousarray(np.asarray(embs, dtype=np.float32))
    ids = np.asarray(idtys).astype(np.int64)
    plans, nt = _plan(ids)
    NPX = nt * 128
    emT = embs.T
    ohN = np.zeros((64, B), np.float32)
    ohN[ids, np.arange(B)] = 1.0
    in_maps = []
    for c in range(NCORES):
        a0 = (c // 2) * AH
        pairs = plans[c]
        npc = len(pairs)
        la = np.zeros(NPX, np.int64)
        pp = np.full(NPX, a0, np.int64)
        if npc:
            la[:npc] = [q[0] for q in pairs]
            pp[:npc] = [q[1] for q in pairs]

        emTT = np.concatenate([emT, emT[:, a0 : a0 + AH]], axis=1)

        fold = np.zeros((128, B + AH), np.float32)
        fold[0:64, 0:B] = ohN                      # rhs: ohN
        ohA = np.zeros((64, AH), np.float32)
        ohA[ids[a0 : a0 + AH], np.arange(AH)] = BIGSQ
        fold[0:64, B : B + AH] = ohA               # lhsT: BIGSQ*ohA

        appp = np.concatenate([embs[a0 + la].T, embs[pp].T], axis=1)

        wvm = np.zeros((AH, NPX + nt), np.float32)
        wvm[la, np.arange(NPX)] = 1.0
        vm = np.zeros(NPX, np.float32)
        vm[:npc] = 1.0
        wvm[:, NPX : NPX + nt] = vm.reshape(nt, 128).T

        in_maps.append(
            {
                "emTT": np.ascontiguousarray(emTT.astype(bf)),
                "fold": np.ascontiguousarray(fold.astype(bf)),
                "appp": np.ascontiguousarray(appp.astype(bf)),
                "wvm": np.ascontiguousarray(wvm.astype(bf)),
            }
        )
    return in_maps, nt


def combine(results, nt):
    total = 0.0
    count = 0.0
    for r in results:
        o = np.asarray(r["out"], dtype=np.float64)
        total += o[:, 0:nt].sum()
        count += o[:, nt:].sum()
    loss = np.float32(total / (count + 1e-16))
    return np.array(loss, dtype=np.float32)


def kernel(embs: np.ndarray, idtys: np.ndarray) -> np.ndarray:
    from concourse import bass_utils

    in_maps, nt = make_in_maps(np.asarray(embs), np.asarray(idtys))
    nc = _get_nc(nt)
    res = bass_utils.run_bass_kernel_spmd(nc, in_maps, list(range(NCORES)))
    return combine(res.results, nt)
